# revision 11
# baseline (speedup 1.0000x reference)
import os
import sys
from contextlib import ExitStack

for _p in ("/opt/trn_rl_repo", "/root/.axon_site/_ro/trn_rl_repo"):
    if os.path.isdir(_p) and _p not in sys.path:
        sys.path.insert(0, _p)

import numpy as np

import bass_rust
import concourse.bacc as bacc
import concourse.mybir as mybir
from concourse.bass_utils import run_bass_kernel_spmd
from concourse.tile import TileContext

F32 = mybir.dt.float32
CMP = mybir.dt.float16  # 2-byte dtype => DVE 2x mode; fp16 passes precision

# bit i of the LBP code corresponds to OFFSETS[i] (same order as reference)
OFFSETS = ((-1, -1), (-1, 0), (-1, 1), (0, 1), (1, 1), (1, 0), (1, -1), (0, -1))

N_CORES = 8
IMG_PER_CORE = 8
H = W = 512
NT = 4          # subrows per partition; image row r = p*4 + t
NBLK = 128      # 8-pixel chunks per half-image batch
NS = 32         # feature slots: 0-15 lo nibble, 16-31 hi nibble
# F free layout: [blk, slot, i] -> offset = blk*256 + slot*8 + i
FP = NBLK * NS * 8  # per-partition elements of an F tile (32768)
GP = NT * (W + 2)   # per-partition elements of gray (2056)

SUBSETS = [(), (0,), (1,), (2,), (3,), (0, 1), (0, 2), (0, 3), (1, 2), (1, 3),
           (2, 3), (0, 1, 2), (0, 1, 3), (0, 2, 3), (1, 2, 3), (0, 1, 2, 3)]

# folded grayscale (compares are scale-invariant): R + C_G*G + C_B*B
C_G = 0.587 / 0.2989
C_B = 0.114 / 0.2989

_NC_CACHE = {}


def _ap(base, dims, offset):
    c = base.copy()
    c.ap = bass_rust.VecI64Pair(dims)
    c.offset = offset
    return c


def _build(n_img=IMG_PER_CORE, reps=1):
    """LBP subset-moment kernel for one core: x [n_img,3,512,512] -> gram
    [n_img,128,128].

    Image row r lives at (partition r//4, subrow t=r%4), so 3 of 4 row
    shifts are free-dim offsets; the p+-1 cases use small DMA row copies
    (engines are 128-lane lockstep).  Grayscale: DVE f32 STT, then ACT
    centers around 0 while converting to fp16 (halves compare flips).
    Compares run as multi-slot overlap-AP tensor_tensor ops at DVE 2x; the
    11 subset products per nibble are multi-slot fp16 multiplies written
    directly into the matmul-ready [blk,32,8] layout; PE accumulates
    Gram(F_lo, F_hi) into [128,128] PSUM.  Host Mobius-inverts the subset
    moments into the 256-bin histogram.
    """
    nc = bacc.Bacc(None, target_bir_lowering=False, debug=False)
    x = nc.dram_tensor("x", [n_img, 3, H, W], F32, kind="ExternalInput")
    out = nc.dram_tensor("gram", [n_img, 128, 128], F32, kind="ExternalOutput")

    GE = mybir.AluOpType.is_ge
    MUL = mybir.AluOpType.mult
    ADD = mybir.AluOpType.add

    with TileContext(nc) as tc, ExitStack() as ctx:
        cpool = ctx.enter_context(tc.tile_pool(name="const", bufs=1))
        ones = cpool.tile([128, 8], CMP)
        nc.vector.memset(ones[:], 1.0)
        biasT = cpool.tile([128, 1], F32)
        nc.vector.memset(biasT[:], -1.675)

        xpool = ctx.enter_context(tc.tile_pool(name="x", bufs=1))
        tpool = ctx.enter_context(tc.tile_pool(name="tmp", bufs=1))
        gpool = ctx.enter_context(tc.tile_pool(name="gray", bufs=2))
        fpool = ctx.enter_context(tc.tile_pool(name="feat", bufs=2))
        hpool = ctx.enter_context(tc.tile_pool(name="hist", bufs=2, space="PSUM"))
        rpool = ctx.enter_context(tc.tile_pool(name="red", bufs=2))

        for img in [i for _ in range(reps) for i in range(n_img)]:
            # ---- load + grayscale ----
            xts = []
            for ch in range(3):
                xt = xpool.tile([128, NT, W], F32, tag=f"x{ch}")
                nc.sync.dma_start(
                    xt[:], x[img, ch].rearrange("(p t) w -> p t w", t=NT))
                xts.append(xt)
            gray = gpool.tile([128, NT, W + 2], CMP, tag="gray")
            tmp = tpool.tile([128, NT, W], F32, tag="gs")
            nc.vector.scalar_tensor_tensor(
                tmp[:], xts[1][:], C_G, xts[0][:], op0=MUL, op1=ADD)
            nc.vector.scalar_tensor_tensor(
                tmp[:], xts[2][:], C_B, tmp[:], op0=MUL, op1=ADD)
            nc.scalar.activation(
                gray[:, :, 1:W + 1], tmp[:],
                mybir.ActivationFunctionType.Identity, bias=biasT[:])
            # replicate-pad columns (tiny)
            nc.vector.tensor_copy(gray[:, :, 0:1], gray[:, :, 1:2])
            nc.vector.tensor_copy(gray[:, :, W + 1:W + 2], gray[:, :, W:W + 1])
            # row-shifted halo copies for the p+-1 subrows:
            # gup[p,:] = row p*4-1 = gray[p-1,t=3]; gdn[p,:] = row p*4+4
            gup = gpool.tile([128, W + 2], CMP, tag="gup")
            gdn = gpool.tile([128, W + 2], CMP, tag="gdn")
            nc.sync.dma_start(gup[1:128, :], gray[0:127, 3, :])
            nc.sync.dma_start(gup[0:1, :], gray[0:1, 0, :])
            nc.sync.dma_start(gdn[0:127, :], gray[1:128, 0, :])
            nc.sync.dma_start(gdn[127:128, :], gray[127:128, 3, :])

            hist = hpool.tile([128, 128], F32)
            first_mm = True
            for half in range(2):
                t0 = half * 2
                F = fpool.tile([128, NBLK, NS, 8], CMP, tag="F")
                Fb = F[:]
                gb = gray[:]

                # ---- compares: 6 multi-slot overlap-AP ops per batch ----
                for tb, t in enumerate((t0, t0 + 1)):
                    bo = tb * 64 * 256  # blk offset of this subrow
                    cent3 = _ap(gb, [[GP, 128], [0, 3], [8, 64], [1, 8]],
                                t * (W + 2) + 1)
                    cent2 = _ap(gb, [[GP, 128], [0, 2], [8, 64], [1, 8]],
                                t * (W + 2) + 1)
                    # dy=-1 -> lo slots 1..3 (dx -1,0,1)
                    o = _ap(Fb, [[FP, 128], [8, 3], [256, 64], [1, 8]], bo + 8)
                    if t > 0:
                        i0 = _ap(gb, [[GP, 128], [1, 3], [8, 64], [1, 8]],
                                 (t - 1) * (W + 2))
                    else:
                        i0 = _ap(gup[:], [[W + 2, 128], [1, 3], [8, 64],
                                          [1, 8]], 0)
                    nc.vector.tensor_tensor(o, i0, cent3, op=GE)
                    # dy=+1 -> hi slots 17..19 (dx 1,0,-1)
                    o = _ap(Fb, [[FP, 128], [8, 3], [256, 64], [1, 8]],
                            bo + 17 * 8)
                    if t < NT - 1:
                        i0 = _ap(gb, [[GP, 128], [-1, 3], [8, 64], [1, 8]],
                                 (t + 1) * (W + 2) + 2)
                    else:
                        i0 = _ap(gdn[:], [[W + 2, 128], [-1, 3], [8, 64],
                                          [1, 8]], 2)
                    nc.vector.tensor_tensor(o, i0, cent3, op=GE)
                    # dy=0 -> slots {4, 20} (dx +1, -1)
                    o = _ap(Fb, [[FP, 128], [16 * 8, 2], [256, 64], [1, 8]],
                            bo + 4 * 8)
                    i0 = _ap(gb, [[GP, 128], [-2, 2], [8, 64], [1, 8]],
                             t * (W + 2) + 2)
                    nc.vector.tensor_tensor(o, i0, cent2, op=GE)

                # ---- constant-1 slots {0,16} (copy, 4x mode) ----
                o = _ap(Fb, [[FP, 128], [16 * 8, 2], [256, 128], [1, 8]], 0)
                i = _ap(ones[:], [[8, 128], [0, 2], [0, 128], [1, 8]], 0)
                nc.vector.tensor_copy(o, i)

                # ---- subset products ----
                def prod(s_in0, s_in1a, j, s_out, B=None):
                    if B is None:  # merged across both nibbles (j==1)
                        o = _ap(Fb, [[FP, 128], [128, 2], [256, NBLK], [1, 8]],
                                s_out * 8)
                        i0 = _ap(Fb, [[FP, 128], [128, 2], [256, NBLK],
                                      [1, 8]], s_in0 * 8)
                        i1 = _ap(Fb, [[FP, 128], [128, 2], [256, NBLK],
                                      [1, 8]], s_in1a * 8)
                    else:
                        o = _ap(Fb, [[FP, 128], [8, j], [256, NBLK], [1, 8]],
                                (B + s_out) * 8)
                        i0 = _ap(Fb, [[FP, 128], [0, j], [256, NBLK], [1, 8]],
                                 (B + s_in0) * 8)
                        i1 = _ap(Fb, [[FP, 128], [8, j], [256, NBLK], [1, 8]],
                                 (B + s_in1a) * 8)
                    nc.vector.tensor_tensor(o, i0, i1, op=MUL)

                for B in (0, 16):
                    prod(1, 2, 3, 5, B)    # b0*{b1,b2,b3} -> s5..s7
                    prod(2, 3, 2, 8, B)    # b1*{b2,b3}    -> s8,s9
                for B in (0, 16):
                    prod(5, 3, 2, 11, B)   # b0b1*{b2,b3}  -> s11,s12
                prod(3, 4, 1, 10)          # b2*b3 -> s10 (both nibbles)
                for B in (0, 16):
                    prod(10, 1, 2, 13, B)  # b2b3*{b0,b1}  -> s13,s14
                prod(11, 4, 1, 15)         # b0b1b2*b3 -> s15 (both nibbles)

                # ---- Gram accumulation on PE ----
                for blk in range(NBLK):
                    last = (half == 1) and (blk == NBLK - 1)
                    nc.tensor.matmul(
                        hist[:],
                        F[:, blk, 0:16, :],
                        F[:, blk, 16:32, :],
                        start=first_mm, stop=last,
                        skip_group_check=True)
                    first_mm = False

            hsb = rpool.tile([128, 128], F32, tag="hsb")
            nc.scalar.copy(hsb[:], hist[:])
            nc.sync.dma_start(out[img, :, :], hsb[:])

    nc.finalize()
    return nc


def _get_nc(key, n_img):
    if key not in _NC_CACHE:
        _NC_CACHE[key] = _build(n_img)
    return _NC_CACHE[key]


def _moebius_matrix():
    A = np.zeros((16, 16), dtype=np.float64)
    for l in range(16):
        lbits = {j for j in range(4) if (l >> j) & 1}
        for s, S in enumerate(SUBSETS):
            if lbits.issubset(S):
                A[l, s] = (-1.0) ** (len(S) - len(lbits))
    return A


_A = _moebius_matrix()

_LAST = {"exec_ns": None, "trace": None}


def kernel(x, _trace=False):
    x = np.ascontiguousarray(np.asarray(x), dtype=np.float32)
    bs = x.shape[0]
    n_img = bs // N_CORES
    nc = _get_nc(("v5", n_img), n_img)
    in_maps = [{"x": x[i * n_img:(i + 1) * n_img]} for i in range(N_CORES)]
    res = run_bass_kernel_spmd(
        nc, in_maps, list(range(N_CORES)), trace=_trace)
    if _trace:
        _LAST["exec_ns"] = res.exec_time_ns
        _LAST["trace"] = res.instructions_and_trace
    gram = np.concatenate(
        [res.results[i]["gram"] for i in range(N_CORES)], axis=0)
    # diagonal [16,16] blocks: M16[s,t] = sum_i gram[s*8+i, t*8+i]
    M = np.einsum("bsitj,ij->bst", gram.reshape(bs, 16, 8, 16, 8).astype(
        np.float64), np.eye(8))
    N = np.einsum("ls,bst,ht->blh", _A, M, _A)  # [bs, lo, hi] counts
    hist = N.transpose(0, 2, 1).reshape(bs, 256).astype(np.float32)
    norm = np.sqrt((hist * hist).sum(axis=1, keepdims=True))
    return (hist / (norm + 1e-6)).astype(np.float32)
